# revision 23
# baseline (speedup 1.0000x reference)
"""NeighborAttentionLayer Trainium2 kernel (8-core data-parallel SPMD).

Strategy
--------
Data-parallel over the batch dim B=64: each of the 8 NeuronCores runs the
full transformer layer for 8 batches (1024 tokens). No collectives.

Mixed-precision matmuls, tuned so the final max-rel error stays under the
2e-2 gate (error budget allocated by an offline quantization simulator):
  * V projection: full fp8(e4m3) with perf_mode=DoubleRow (2 contraction
    tiles per matmul, ~1.4x bf16 rate)
  * Q/K projection: first N8QK of 20 contraction k-tiles fp8-DoubleRow,
    rest bf16 (the bf16 weights carry an exact x1024 exponent-shift so
    both parts accumulate into one PSUM at the same scale)
  * out_proj: full fp8-DoubleRow
  * FFN1/FFN2: bf16 (worst error-per-saved-ns sites, kept full precision)

All descales fold into existing ops: the PSUM->SBUF copy activations take
a `scale`, the V descale rides the attnT transpose-copy (v_sb holds the
raw x1024 PSUM value), and the out_proj descale folds into a pre-scaled x
residual (LayerNorm is scale-invariant; its eps is scaled to match).

Attention output (aoT) stays in SBUF (20KB/partition) instead of a DRAM
round-trip. LN1/LN2 for each token tile run inside the last out-proj /
FFN2 chunk iteration so the vector-engine LayerNorm work overlaps the
next tile's matmuls instead of serializing at phase end.

Host-side prep (numpy, not on HW): weight transpose/scale/cast/tiling,
head-pair-interleaved q/k feature order (each head's 320 features map
onto 128-partition tiles as 128+128+64 slices), x shipped as fp32
residual (pre-scaled), fp8 transpose, and bf16 transpose (qk bf16 part).

The learned distance-bias MLP adds a per-query bias broadcast over keys;
softmax over keys is invariant to it, so it is skipped. The key-padding
mask is all-ones per the problem spec (fill=ones); a non-trivial mask is
applied multiplicatively on the exp'd scores.
"""

import numpy as np
import ml_dtypes

# ---- problem constants (hardcoded per contract) ----
B, K, D, H, DFF = 64, 128, 2560, 8, 1024
HD = D // H                    # 320
EPS = 1e-5
NCORES = 8
BL = B // NCORES               # 8 batches per core
TOK = BL * K                   # 1024 tokens per core
P = 128
DT = D // P                    # 20 d-tiles
FT = DFF // P                  # 8 dff-tiles
CH = 512                       # matmul moving-dim chunk (psum bank limit)
NHALF = 2                      # token halves for attention SBUF pressure
THALF = TOK // NHALF           # 512 tokens per half
BHALF = BL // NHALF            # 4 batches per half
QKT = 2 * DT                   # 40 q+k feature tiles

# ---- precision config (validated by err_sim2.py offline simulator) ----
FP8_V = True                   # v projection fp8 DoubleRow (all k-tiles)
N8QK = 8                       # q/k projection: first N8QK k-tiles fp8 (even)
FP8_OUT = True                 # out_proj fp8 DoubleRow
SX = 16.0                      # x fp8 pre-scale
SW = 64.0                      # weight fp8 pre-scale
SAO = 16.0                     # attention-output fp8 pre-scale
FP8_MAX = 240.0                # TRN e4m3 max normal

VRAW = SX * SW if FP8_V else 1.0      # scale carried by v_sb
S1 = SAO * SW if FP8_OUT else 1.0     # out_proj psum / y / x-residual scale
S2 = 1.0                              # x1 storage scale (ffn2 is bf16)


def _qk_perm():
    """Head-pair interleaved feature order for q (and k) projections."""
    perm = []
    for p in range(H // 2):
        h0, h1 = 2 * p, 2 * p + 1
        perm.extend(range(HD * h0, HD * h0 + 256))         # tiles 5p+0, 5p+1
        perm.extend(range(HD * h0 + 256, HD * h0 + 320))   # tile 5p+2 lo
        perm.extend(range(HD * h1 + 256, HD * h1 + 320))   # tile 5p+2 hi
        perm.extend(range(HD * h1, HD * h1 + 256))         # tiles 5p+3, 5p+4
    return np.array(perm)


def _score_ktiles(h):
    """(tile, row0, row1) triples (within the 20 q-tiles) contracting head h."""
    p = h // 2
    if h % 2 == 0:
        return [(5 * p + 0, 0, 128), (5 * p + 1, 0, 128), (5 * p + 2, 0, 64)]
    return [(5 * p + 3, 0, 128), (5 * p + 4, 0, 128), (5 * p + 2, 64, 128)]


def _ao_segments():
    """Per d-tile (real feature order) segments for attn@V:
    list over tiles of [(head, d0, d1, psum_base), ...]."""
    segs = [[] for _ in range(DT)]
    for h in range(H):
        d = HD * h
        end = HD * (h + 1)
        while d < end:
            nxt = min(end, (d // P + 1) * P)
            segs[d // P].append((h, d, nxt, d % P))
            d = nxt
    return segs


def _tileize_f32(wT, chunk):
    """[Kin, N] -> [N/chunk, 128, Kin/128, chunk] contiguous fp32 blocks."""
    kin, n = wT.shape
    return np.ascontiguousarray(
        wT.reshape(kin // P, P, n // chunk, chunk).transpose(2, 1, 0, 3))


def _fp8(a):
    return np.clip(a, -FP8_MAX, FP8_MAX).astype(ml_dtypes.float8_e4m3)


def build_core_program(use_qk_bias, use_v_bias, use_out_bias, use_b1, use_b2,
                       ln1_affine, ln2_affine, use_mask):
    import concourse.bass as bass
    import concourse.bacc as bacc
    import concourse.mybir as mybir
    import concourse.tile as tile
    from concourse.masks import make_identity

    F32 = mybir.dt.float32
    BF16 = mybir.dt.bfloat16
    F8 = mybir.dt.float8e4
    DR = mybir.MatmulPerfMode.DoubleRow

    V_DT = F8 if FP8_V else BF16
    OUT_DT = F8 if FP8_OUT else BF16

    k_desc = 1.0 / (SX * SW)          # q/k psum descale (both parts x1024)
    q_desc = k_desc / np.sqrt(HD)

    nc = bacc.Bacc()
    dp = nc.declare_dram_parameter
    xT8 = dp("xT8", [NHALF, P, DT, THALF], F8, isOutput=False)
    xTb = dp("xTb", [NHALF, P, DT - N8QK, THALF], BF16, isOutput=False)
    x_nat = dp("x", [TOK, D], F32, isOutput=False)
    qk_wT8 = dp("qk_wT8", [QKT, P, N8QK, P], F8, isOutput=False)
    qk_wTb = dp("qk_wTb", [QKT, P, DT - N8QK, P], BF16, isOutput=False)
    v_wT = dp("v_wT", [D // CH, P, DT, CH], V_DT, isOutput=False)
    out_wT = dp("out_wT", [D // CH, P, DT, CH], OUT_DT, isOutput=False)
    w1T = dp("w1T", [FT, P, DT, P], BF16, isOutput=False)
    w2T = dp("w2T", [D // CH, P, FT, CH], BF16, isOutput=False)
    qk_b = dp("qk_b", [2 * D], F32, isOutput=False) if use_qk_bias else None
    v_b = dp("v_b", [D], F32, isOutput=False) if use_v_bias else None
    out_b = dp("out_b", [D], F32, isOutput=False) if use_out_bias else None
    b1 = dp("b1", [DFF], F32, isOutput=False) if use_b1 else None
    b2 = dp("b2", [D], F32, isOutput=False) if use_b2 else None
    ln1_g = dp("ln1_g", [D], F32, isOutput=False) if ln1_affine else None
    ln1_b = dp("ln1_b", [D], F32, isOutput=False) if ln1_affine else None
    ln2_g = dp("ln2_g", [D], F32, isOutput=False) if ln2_affine else None
    ln2_b = dp("ln2_b", [D], F32, isOutput=False) if ln2_affine else None
    mask_in = dp("mask", [BL, K], F32, isOutput=False) if use_mask else None
    out = dp("out", [TOK, D], F32, isOutput=True)

    Exp = mybir.ActivationFunctionType.Exp
    Relu = mybir.ActivationFunctionType.Relu
    Sqrt = mybir.ActivationFunctionType.Sqrt
    Copy = mybir.ActivationFunctionType.Copy
    Ident = mybir.ActivationFunctionType.Identity
    AX = mybir.AxisListType.X
    OP = mybir.AluOpType

    def bcast_dram(ap, n_part=P):
        return bass.AP(tensor=ap.tensor, offset=ap.offset,
                       ap=[[0, n_part]] + list(ap.ap))

    ao_segs = _ao_segments()

    with tile.TileContext(nc) as tc:
        with (
            tc.tile_pool(name="consts", bufs=1) as consts,
            tc.tile_pool(name="aot", bufs=1) as aot,
        ):
            id_bf = consts.tile([P, P], BF16)
            make_identity(nc, id_bf)
            id_f32 = consts.tile([P, P], F32)
            make_identity(nc, id_f32)
            # LN eps consts absorbing activation scaling: LN1 computes
            # sqrt(var_s/S2^2 + EPS*S1^2/S2^2), LN2 sqrt(var_s + EPS*S2^2)
            eps1_sb = consts.tile([P, 1], F32)
            nc.vector.memset(eps1_sb, EPS * (S1 * S1) / (S2 * S2))
            eps2_sb = consts.tile([P, 1], F32)
            nc.vector.memset(eps2_sb, EPS * (S2 * S2))

            # first out_proj weight chunk, preloaded so phase B starts hot
            # (gpsimd queue: must not delay the startup xT8/wv loads on sync)
            wo_first = consts.tile([P, DT, CH], OUT_DT)
            nc.gpsimd.dma_start(out=wo_first, in_=out_wT[0])

            # attention output transposed, resident in SBUF (no DRAM trip)
            aoT_sb = aot.tile([P, BL, DT, P], OUT_DT)

            qkb_sb = None
            if use_qk_bias:
                qkb_sb = consts.tile([P, QKT], F32)
                nc.sync.dma_start(out=qkb_sb,
                                  in_=qk_b[:].rearrange("(t p) -> p t", p=P))
            vb_sb = None
            if use_v_bias:
                # v_b host-scaled by VRAW (v_sb carries VRAW*v)
                vb_sb = consts.tile([P, D], F32)
                nc.gpsimd.dma_start(out=vb_sb, in_=bcast_dram(v_b[:]))
            outb_sb = None
            if use_out_bias:
                outb_sb = consts.tile([P, D], F32)
                nc.gpsimd.dma_start(out=outb_sb, in_=bcast_dram(out_b[:]))
            b1_sb = None
            if use_b1:
                b1_sb = consts.tile([P, FT], F32)
                nc.sync.dma_start(out=b1_sb,
                                  in_=b1[:].rearrange("(t p) -> p t", p=P))
            b2_sb = None
            if use_b2:
                b2_sb = consts.tile([P, D], F32)
                nc.gpsimd.dma_start(out=b2_sb, in_=bcast_dram(b2[:]))
            ln1g_sb = ln1b_sb = ln2g_sb = ln2b_sb = None
            if ln1_affine:
                ln1g_sb = consts.tile([P, D], F32)
                nc.gpsimd.dma_start(out=ln1g_sb, in_=bcast_dram(ln1_g[:]))
                ln1b_sb = consts.tile([P, D], F32)
                nc.gpsimd.dma_start(out=ln1b_sb, in_=bcast_dram(ln1_b[:]))
            if ln2_affine:
                ln2g_sb = consts.tile([P, D], F32)
                nc.gpsimd.dma_start(out=ln2g_sb, in_=bcast_dram(ln2_g[:]))
                ln2b_sb = consts.tile([P, D], F32)
                nc.gpsimd.dma_start(out=ln2b_sb, in_=bcast_dram(ln2_b[:]))
            mask_sb = None
            if use_mask:
                mask_sb = consts.tile([P, BL, K], F32)
                nc.gpsimd.dma_start(
                    out=mask_sb, in_=bcast_dram(mask_in[:, :]))

            # ======== attention: both halves share one set of buffers ========
            with (
                tc.tile_pool(name="attn_sb", bufs=1) as asb,
                tc.tile_pool(name="aw", bufs=3) as aw,
                tc.tile_pool(name="bt", bufs=2) as bt,
            ):
                xT8_sb = asb.tile([P, DT, THALF], F8)
                xTb_sb = asb.tile([P, DT - N8QK, THALF], BF16)
                v_sb = asb.tile([P, BHALF, D], BF16)
                qkT_sb = asb.tile([P, QKT, THALF], BF16)

                for half in range(NHALF):
                    nc.sync.dma_start(out=xT8_sb, in_=xT8[half])
                    nc.sync.dma_start(out=xTb_sb, in_=xTb[half])

                    with tc.tile_pool(name=f"aps{half}", bufs=4,
                                      space="PSUM") as aps:
                        # V projection: natural [tok, vfeat]; v_sb holds the
                        # RAW psum (xVRAW); descale rides the attnT copy.
                        for c in range(D // CH):
                            wv = aw.tile([P, DT, CH], V_DT, tag="wv")
                            nc.sync.dma_start(out=wv, in_=v_wT[c])
                            for t in range(BHALF):
                                ps = aps.tile([P, CH], F32, tag="ps_a")
                                if FP8_V:
                                    for k in range(DT // 2):
                                        nc.tensor.matmul(
                                            ps,
                                            xT8_sb[:, 2 * k:2 * k + 2,
                                                   t * P:(t + 1) * P],
                                            wv[:, 2 * k:2 * k + 2, :],
                                            start=(k == 0),
                                            stop=(k == DT // 2 - 1),
                                            perf_mode=DR)
                                else:
                                    for k in range(DT):
                                        nc.tensor.matmul(
                                            ps,
                                            xTb_sb[:, k - N8QK,
                                                   t * P:(t + 1) * P],
                                            wv[:, k, :],
                                            start=(k == 0), stop=(k == DT - 1))
                                vdst = v_sb[:, t, c * CH:(c + 1) * CH]
                                if use_v_bias:
                                    nc.vector.tensor_add(
                                        out=vdst, in0=ps,
                                        in1=vb_sb[:, c * CH:(c + 1) * CH])
                                else:
                                    nc.vector.tensor_copy(out=vdst, in_=ps)

                        # Q/K projection: transposed [feat, tok]; first N8QK
                        # k-tiles fp8-DR, rest bf16 (weights carry x1024)
                        for jt in range(QKT):
                            wq8 = aw.tile([P, N8QK, P], F8, tag="wq8")
                            nc.sync.dma_start(out=wq8, in_=qk_wT8[jt])
                            wqb = aw.tile([P, DT - N8QK, P], BF16, tag="wqb")
                            nc.sync.dma_start(out=wqb, in_=qk_wTb[jt])
                            ps = aps.tile([P, CH], F32, tag="ps_a")
                            for k in range(N8QK // 2):
                                nc.tensor.matmul(
                                    ps, wq8[:, 2 * k:2 * k + 2, :],
                                    xT8_sb[:, 2 * k:2 * k + 2, :],
                                    start=(k == 0), stop=False,
                                    perf_mode=DR)
                            for k in range(N8QK, DT):
                                nc.tensor.matmul(
                                    ps, wqb[:, k - N8QK, :],
                                    xTb_sb[:, k - N8QK, :],
                                    start=False, stop=(k == DT - 1))
                            desc = q_desc if jt < DT else k_desc
                            if use_qk_bias:
                                nc.scalar.activation(
                                    out=qkT_sb[:, jt, :], in_=ps, func=Ident,
                                    bias=qkb_sb[:, jt:jt + 1], scale=desc)
                            else:
                                nc.scalar.activation(out=qkT_sb[:, jt, :],
                                                     in_=ps, func=Copy,
                                                     scale=desc)

                    # attention per batch: scores -> transposes -> attn@V,
                    # each stage contiguous on PE so no mid-stream waits
                    with (
                        tc.tile_pool(name=f"sps{half}", bufs=4,
                                     space="PSUM") as sps,
                        tc.tile_pool(name=f"tps{half}", bufs=2,
                                     space="PSUM") as tps,
                        tc.tile_pool(name=f"ops{half}", bufs=2,
                                     space="PSUM") as ops,
                    ):
                        for bi in range(BHALF):
                            b = half * BHALF + bi
                            csl = slice(bi * P, (bi + 1) * P)
                            attn = bt.tile([P, H, P], BF16, tag="attn")
                            negmax = bt.tile([P, H], F32, tag="negmax")
                            esum = bt.tile([P, H], F32, tag="esum")
                            rinv = bt.tile([P, H], F32, tag="rinv")
                            attnT = bt.tile([P, H, P], BF16, tag="attnT")
                            for h in range(H):
                                sc = sps.tile([P, P], F32, tag="sc")
                                kts = _score_ktiles(h)
                                for i, (t, r0, r1) in enumerate(kts):
                                    nc.tensor.matmul(
                                        sc, qkT_sb[r0:r1, t, csl],
                                        qkT_sb[r0:r1, DT + t, csl],
                                        start=(i == 0), stop=(i == len(kts) - 1))
                                nc.vector.tensor_reduce(
                                    out=negmax[:, h:h + 1], in_=sc, axis=AX,
                                    op=OP.max, negate=True)
                                nc.scalar.activation(
                                    out=attn[:, h, :], in_=sc, func=Exp,
                                    bias=negmax[:, h:h + 1], scale=1.0,
                                    accum_out=esum[:, h:h + 1])
                                if use_mask:
                                    nc.vector.tensor_mul(
                                        out=attn[:, h, :], in0=attn[:, h, :],
                                        in1=mask_sb[:, b, :])
                                    nc.vector.tensor_reduce(
                                        out=esum[:, h:h + 1], in_=attn[:, h, :],
                                        axis=AX, op=OP.add)
                                nc.vector.reciprocal(out=rinv[:, h:h + 1],
                                                     in_=esum[:, h:h + 1])
                                nc.vector.tensor_scalar_mul(
                                    out=attn[:, h, :], in0=attn[:, h, :],
                                    scalar1=rinv[:, h:h + 1])
                            for h in range(H):
                                tp = tps.tile([P, P], BF16, tag="tp")
                                nc.tensor.transpose(tp, attn[:, h, :], id_bf)
                                # attnT carries attn/VRAW: cancels v_sb's
                                # VRAW so ao psum is true-scale
                                nc.vector.tensor_scalar_mul(
                                    out=attnT[:, h, :], in0=tp,
                                    scalar1=1.0 / VRAW)
                            for t in range(DT):
                                ao = ops.tile([P, P], F32, tag="ao")
                                for (h, d0, d1, base) in ao_segs[t]:
                                    w = d1 - d0
                                    nc.tensor.matmul(
                                        ao[base:base + w, :], v_sb[:, bi, d0:d1],
                                        attnT[:, h, :], start=True, stop=True,
                                        tile_position=((0, base) if base
                                                       else None))
                                nc.scalar.activation(
                                    out=aoT_sb[:, b, t, :], in_=ao, func=Copy,
                                    scale=(SAO if FP8_OUT else 1.0))

            # ==== per group: out_proj+LN1 -> FFN1 -> FFN2+LN2 (fused B/C) ====
            # Group g's FFN2 matmuls follow g's FFN1 directly, so the
            # vector/scalar LayerNorm tails overlap the next group's matmuls
            # instead of serializing at phase end. LayerNorm apply runs on
            # the scalar engine (out = y*rstd + (-mean*rstd)).
            NGRP = 2
            TPG = BL // NGRP          # tok-tiles per group
            GW = TPG * P              # tokens per group (512)
            with (
                tc.tile_pool(name="hres", bufs=1) as hres,
                tc.tile_pool(name="csb", bufs=2) as csb,
                tc.tile_pool(name="cxr", bufs=4) as cxr,
                tc.tile_pool(name="cx1", bufs=4) as cx1,
                tc.tile_pool(name="cw", bufs=2) as cw,
                tc.tile_pool(name="cy", bufs=1) as cy,
                tc.tile_pool(name="cx1t", bufs=1) as cx1t,
                tc.tile_pool(name="dw", bufs=2) as dw,
                tc.tile_pool(name="ew", bufs=2) as ew,
                tc.tile_pool(name="cps", bufs=6, space="PSUM") as cps,
                tc.tile_pool(name="ctps", bufs=2, space="PSUM") as ctps,
            ):
                hT = hres.tile([P, FT, TOK], BF16)
                # one resident y buffer: out_proj writes it, LN1 reads it,
                # FFN2 overwrites it (per-slice deps order the reuse)
                y_grp = cy.tile([P, TPG, D], F32)

                def ln_apply(stats_row, eps_sb, sqrt_scale, out_t, y_row,
                             gain_sb, bias_sb):
                    mv = csb.tile([P, 2], F32, tag="mv")
                    nc.vector.bn_aggr(out=mv, in_=stats_row)
                    std = csb.tile([P, 1], F32, tag="std")
                    nc.scalar.activation(out=std, in_=mv[:, 1:2],
                                         func=Sqrt, bias=eps_sb,
                                         scale=sqrt_scale)
                    rstd = csb.tile([P, 1], F32, tag="rstd")
                    nc.vector.reciprocal(out=rstd, in_=std)
                    nc.vector.tensor_scalar(out=out_t, in0=y_row,
                                            scalar1=mv[:, 0:1], scalar2=rstd,
                                            op0=OP.subtract, op1=OP.mult)
                    if gain_sb is not None:
                        nc.vector.tensor_mul(out=out_t, in0=out_t, in1=gain_sb)
                        nc.vector.tensor_add(out=out_t, in0=out_t, in1=bias_sb)

                # one-chunk-ahead prefetch rings for the streamed weights so
                # group boundaries never expose the DMA latency
                NCH = D // CH
                wo_seq = [c for _ in range(NGRP) for c in range(NCH)]
                w2_seq = list(wo_seq)

                def wo_fetch(i):
                    if i >= len(wo_seq):
                        return None
                    t = cw.tile([P, DT, CH], OUT_DT, tag="wo")
                    nc.sync.dma_start(out=t, in_=out_wT[wo_seq[i]])
                    return t

                def w2_fetch(i):
                    if i >= len(w2_seq):
                        return None
                    t = ew.tile([P, FT, CH], BF16, tag="w2c")
                    nc.sync.dma_start(out=t, in_=w2T[w2_seq[i]])
                    return t

                wo_cur, wo_next = wo_first, wo_fetch(1)
                w2_cur, w2_next = w2_fetch(0), w2_fetch(1)

                for g in range(NGRP):
                    stats_g = csb.tile([P, TPG, 5, 6], F32, tag="stats")
                    x1T_grp = cx1t.tile([P, DT, GW], BF16, tag="x1T_grp")
                    x1_keep = [None] * TPG      # group's x1 rows, SBUF-resident
                    # ---- out_proj + residual + LN1 ----
                    for c in range(D // CH):
                        wo = wo_cur
                        wo_cur, wo_next = wo_next, wo_fetch(g * NCH + c + 2)
                        for ti in range(TPG):
                            tt = g * TPG + ti
                            ps = cps.tile([P, CH], F32, tag="ps")
                            if FP8_OUT:
                                for k in range(DT // 2):
                                    nc.tensor.matmul(
                                        ps,
                                        aoT_sb[:, tt, 2 * k:2 * k + 2, :],
                                        wo[:, 2 * k:2 * k + 2, :],
                                        start=(k == 0),
                                        stop=(k == DT // 2 - 1),
                                        perf_mode=DR)
                            else:
                                for k in range(DT):
                                    nc.tensor.matmul(
                                        ps, aoT_sb[:, tt, k, :], wo[:, k, :],
                                        start=(k == 0), stop=(k == DT - 1))
                            if use_out_bias:
                                # out_b host-scaled by S1
                                nc.vector.tensor_add(
                                    out=ps, in0=ps,
                                    in1=outb_sb[:, c * CH:(c + 1) * CH])
                            # deep ring: the trigger's WAR wait looks 4 allocs
                            # back, so it never stalls the sync DMA queue
                            xr = cxr.tile([P, CH], F32, tag="xr")
                            nc.sync.dma_start(
                                out=xr,
                                in_=x_nat[tt * P:(tt + 1) * P,
                                          c * CH:(c + 1) * CH])
                            nc.vector.tensor_add(
                                out=y_grp[:, ti, c * CH:(c + 1) * CH],
                                in0=ps, in1=xr)
                            nc.vector.bn_stats(
                                out=stats_g[:, ti, c, :],
                                in_=y_grp[:, ti, c * CH:(c + 1) * CH])
                            if c != D // CH - 1:
                                continue
                            # ---- LN1(ti), overlaps next tile's matmuls ----
                            # x1 stays in SBUF: it is both the FFN1 input
                            # (transposed below) and the FFN2 residual
                            x1_t = cx1.tile([P, D], F32, tag="x1k")
                            x1_keep[ti] = x1_t
                            ln_apply(stats_g[:, ti], eps1_sb,
                                     1.0 / (S2 * S2), x1_t, y_grp[:, ti, :],
                                     ln1g_sb if ln1_affine else None, ln1b_sb)
                            for k in range(DT):
                                tp = ctps.tile([P, P], F32, tag="tp_c")
                                nc.tensor.transpose(
                                    tp, x1_t[:, k * P:(k + 1) * P], id_f32)
                                nc.scalar.activation(
                                    out=x1T_grp[:, k, ti * P:(ti + 1) * P],
                                    in_=tp, func=Copy, scale=1.0 / S2)
                    # ---- FFN1 (relu, output into hT) ----
                    for ft in range(FT):
                        w1 = dw.tile([P, DT, P], BF16, tag="w1")
                        nc.sync.dma_start(out=w1, in_=w1T[ft])
                        ps = cps.tile([P, CH], F32, tag="ps")
                        for k in range(DT):
                            nc.tensor.matmul(
                                ps, w1[:, k, :], x1T_grp[:, k, :],
                                start=(k == 0), stop=(k == DT - 1))
                        osl = slice(g * GW, (g + 1) * GW)
                        if use_b1:
                            nc.scalar.activation(
                                out=hT[:, ft, osl], in_=ps, func=Relu,
                                bias=b1_sb[:, ft:ft + 1], scale=1.0)
                        else:
                            nc.scalar.activation(out=hT[:, ft, osl],
                                                 in_=ps, func=Relu)
                    # ---- FFN2 + residual + LN2 (reuses y_grp) ----
                    stats_e = csb.tile([P, TPG, 5, 6], F32, tag="stats_e")
                    for c in range(D // CH):
                        w2c = w2_cur
                        w2_cur, w2_next = w2_next, w2_fetch(g * NCH + c + 2)
                        for ti in range(TPG):
                            tt = g * TPG + ti
                            ps = cps.tile([P, CH], F32, tag="ps")
                            for k in range(FT):
                                nc.tensor.matmul(
                                    ps, hT[:, k, tt * P:(tt + 1) * P],
                                    w2c[:, k, :],
                                    start=(k == 0), stop=(k == FT - 1))
                            if use_b2:
                                # b2 host-scaled by S2
                                nc.vector.tensor_add(
                                    out=ps, in0=ps,
                                    in1=b2_sb[:, c * CH:(c + 1) * CH])
                            nc.vector.tensor_add(
                                out=y_grp[:, ti, c * CH:(c + 1) * CH],
                                in0=ps,
                                in1=x1_keep[ti][:, c * CH:(c + 1) * CH])
                            nc.vector.bn_stats(
                                out=stats_e[:, ti, c, :],
                                in_=y_grp[:, ti, c * CH:(c + 1) * CH])
                            if c != D // CH - 1:
                                continue
                            # ---- LN2(ti), overlaps next tile's matmuls ----
                            # o_t rotates through the same ring as x1_keep;
                            # slot ti's last x1 read (the add above) is done
                            o_t = cx1.tile([P, D], F32, tag="x1k")
                            ln_apply(stats_e[:, ti], eps2_sb, 1.0, o_t,
                                     y_grp[:, ti, :],
                                     ln2g_sb if ln2_affine else None, ln2b_sb)
                            nc.gpsimd.dma_start(
                                out=out[tt * P:(tt + 1) * P, :], in_=o_t)

    nc.compile()
    return nc


def _prep_inputs(x, distances, mask, qkv_w, qkv_b, out_w, out_b,
                 bias_w1, bias_b1, bias_w2, bias_b2,
                 ffn_w1, ffn_b1, ffn_w2, ffn_b2,
                 ln1_g, ln1_b, ln2_g, ln2_b):
    """Host-side shard + weight formatting. Returns (flags, in_maps)."""
    bf16 = ml_dtypes.bfloat16
    perm = _qk_perm()

    # q/k: no sqrt(HD) fold (descale handles it); bf16 part carries x1024
    # so both halves of the contraction accumulate at the same psum scale
    q_w = qkv_w[0:D][perm]
    k_w = qkv_w[D:2 * D][perm]
    qk_cat = np.concatenate([q_w, k_w], axis=0).T.astype(np.float32)
    qk_tz = _tileize_f32(qk_cat, P)                 # [40, P, DT, P]
    qk_wT8 = _fp8(qk_tz[:, :, :N8QK] * np.float32(SW))
    qk_wTb = (qk_tz[:, :, N8QK:] * np.float32(SX * SW)).astype(bf16)

    v_tz = _tileize_f32(qkv_w[2 * D:3 * D].T.astype(np.float32), CH)
    if FP8_V:
        v_wT = _fp8(v_tz * np.float32(SW))
    else:
        v_wT = v_tz.astype(bf16)

    o_tz = _tileize_f32(out_w.T.astype(np.float32), CH)
    if FP8_OUT:
        out_wT = _fp8(o_tz * np.float32(SW))
    else:
        out_wT = o_tz.astype(bf16)
    w1T = _tileize_f32(ffn_w1.T.astype(np.float32), P).astype(bf16)
    w2T = _tileize_f32(ffn_w2.T.astype(np.float32), CH).astype(bf16)

    qk_b = np.concatenate([qkv_b[0:D][perm] * np.float32(1.0 / np.sqrt(HD)),
                           qkv_b[D:2 * D][perm]]).astype(np.float32)
    v_b = np.ascontiguousarray(qkv_b[2 * D:3 * D]).astype(np.float32)

    flags = dict(
        use_qk_bias=bool(np.any(qk_b != 0)),
        use_v_bias=bool(np.any(v_b != 0)),
        use_out_bias=bool(np.any(out_b != 0)),
        use_b1=bool(np.any(ffn_b1 != 0)),
        use_b2=bool(np.any(ffn_b2 != 0)),
        ln1_affine=not (np.all(ln1_g == 1) and np.all(ln1_b == 0)),
        ln2_affine=not (np.all(ln2_g == 1) and np.all(ln2_b == 0)),
        use_mask=not bool(np.all(mask)),
    )

    shared = {"qk_wT8": qk_wT8, "qk_wTb": qk_wTb, "v_wT": v_wT,
              "out_wT": out_wT, "w1T": w1T, "w2T": w2T}
    if flags["use_qk_bias"]:
        shared["qk_b"] = qk_b
    if flags["use_v_bias"]:
        shared["v_b"] = (v_b * np.float32(VRAW)).astype(np.float32)
    if flags["use_out_bias"]:
        shared["out_b"] = (out_b * np.float32(S1)).astype(np.float32)
    if flags["use_b1"]:
        shared["b1"] = ffn_b1.astype(np.float32)
    if flags["use_b2"]:
        shared["b2"] = (ffn_b2 * np.float32(S2)).astype(np.float32)
    if flags["ln1_affine"]:
        shared["ln1_g"] = ln1_g.astype(np.float32)
        shared["ln1_b"] = (ln1_b * np.float32(S2)).astype(np.float32)
    if flags["ln2_affine"]:
        shared["ln2_g"] = ln2_g.astype(np.float32)
        shared["ln2_b"] = ln2_b.astype(np.float32)

    in_maps = []
    for c in range(NCORES):
        xc = np.ascontiguousarray(
            x[c * BL:(c + 1) * BL].reshape(TOK, D)).astype(np.float32)
        xcT = xc.T                                   # [D, TOK]
        xT8_blocks = np.ascontiguousarray(
            _fp8(xcT * np.float32(SX)).reshape(
                DT, P, NHALF, THALF).transpose(2, 1, 0, 3))
        xTb_blocks = np.ascontiguousarray(
            xcT.astype(bf16).reshape(
                DT, P, NHALF, THALF)[N8QK:].transpose(2, 1, 0, 3))
        m = {"x": xc * np.float32(S1), "xT8": xT8_blocks, "xTb": xTb_blocks,
             **shared}
        if flags["use_mask"]:
            m["mask"] = mask[c * BL:(c + 1) * BL].astype(np.float32)
        in_maps.append(m)
    return flags, in_maps


def run(trace=False, **inputs):
    """Build + run on 8 cores. Returns (output, BassKernelResults)."""
    from concourse.bass_utils import run_bass_kernel_spmd

    inputs = {k: np.asarray(v) for k, v in inputs.items()}
    flags, in_maps = _prep_inputs(**inputs)
    nc = build_core_program(**flags)
    res = run_bass_kernel_spmd(nc, in_maps, list(range(NCORES)), trace=trace)
    out = np.stack([np.asarray(res.results[c]["out"], dtype=np.float32)
                    for c in range(NCORES)])
    return out.reshape(B, K, D), res


def kernel(**inputs):
    out, _ = run(trace=False, **inputs)
    return out


# revision 24
# speedup vs baseline: 1.0048x; 1.0048x over previous
"""NeighborAttentionLayer Trainium2 kernel (8-core data-parallel SPMD).

Strategy
--------
Data-parallel over the batch dim B=64: each of the 8 NeuronCores runs the
full transformer layer for 8 batches (1024 tokens). No collectives.

Mixed-precision matmuls, tuned so the final max-rel error stays under the
2e-2 gate (error budget allocated by an offline quantization simulator):
  * V projection: full fp8(e4m3) with perf_mode=DoubleRow (2 contraction
    tiles per matmul, ~1.4x bf16 rate)
  * Q/K projection: first N8QK of 20 contraction k-tiles fp8-DoubleRow,
    rest bf16 (the bf16 weights carry an exact x1024 exponent-shift so
    both parts accumulate into one PSUM at the same scale)
  * out_proj: full fp8-DoubleRow
  * FFN1/FFN2: bf16 (worst error-per-saved-ns sites, kept full precision)

All descales fold into existing ops: the PSUM->SBUF copy activations take
a `scale`, the V descale rides the attnT transpose-copy (v_sb holds the
raw x1024 PSUM value), and the out_proj descale folds into a pre-scaled x
residual (LayerNorm is scale-invariant; its eps is scaled to match).

Attention output (aoT) stays in SBUF (20KB/partition) instead of a DRAM
round-trip. LN1/LN2 for each token tile run inside the last out-proj /
FFN2 chunk iteration so the vector-engine LayerNorm work overlaps the
next tile's matmuls instead of serializing at phase end.

Host-side prep (numpy, not on HW): weight transpose/scale/cast/tiling,
head-pair-interleaved q/k feature order (each head's 320 features map
onto 128-partition tiles as 128+128+64 slices), x shipped as fp32
residual (pre-scaled), fp8 transpose, and bf16 transpose (qk bf16 part).

The learned distance-bias MLP adds a per-query bias broadcast over keys;
softmax over keys is invariant to it, so it is skipped. The key-padding
mask is all-ones per the problem spec (fill=ones); a non-trivial mask is
applied multiplicatively on the exp'd scores.
"""

import numpy as np
import ml_dtypes

# ---- problem constants (hardcoded per contract) ----
B, K, D, H, DFF = 64, 128, 2560, 8, 1024
HD = D // H                    # 320
EPS = 1e-5
NCORES = 8
BL = B // NCORES               # 8 batches per core
TOK = BL * K                   # 1024 tokens per core
P = 128
DT = D // P                    # 20 d-tiles
FT = DFF // P                  # 8 dff-tiles
CH = 512                       # matmul moving-dim chunk (psum bank limit)
NHALF = 2                      # token halves for attention SBUF pressure
THALF = TOK // NHALF           # 512 tokens per half
BHALF = BL // NHALF            # 4 batches per half
QKT = 2 * DT                   # 40 q+k feature tiles

# ---- precision config (validated by err_sim2.py offline simulator) ----
FP8_V = True                   # v projection fp8 DoubleRow (all k-tiles)
N8QK = 8                       # q/k projection: first N8QK k-tiles fp8 (even)
FP8_OUT = True                 # out_proj fp8 DoubleRow
SX = 16.0                      # x fp8 pre-scale
SW = 64.0                      # weight fp8 pre-scale
SAO = 16.0                     # attention-output fp8 pre-scale
FP8_MAX = 240.0                # TRN e4m3 max normal

VRAW = SX * SW if FP8_V else 1.0      # scale carried by v_sb
S1 = SAO * SW if FP8_OUT else 1.0     # out_proj psum / y / x-residual scale
S2 = 1.0                              # x1 storage scale (ffn2 is bf16)


def _qk_perm():
    """Head-pair interleaved feature order for q (and k) projections."""
    perm = []
    for p in range(H // 2):
        h0, h1 = 2 * p, 2 * p + 1
        perm.extend(range(HD * h0, HD * h0 + 256))         # tiles 5p+0, 5p+1
        perm.extend(range(HD * h0 + 256, HD * h0 + 320))   # tile 5p+2 lo
        perm.extend(range(HD * h1 + 256, HD * h1 + 320))   # tile 5p+2 hi
        perm.extend(range(HD * h1, HD * h1 + 256))         # tiles 5p+3, 5p+4
    return np.array(perm)


def _score_ktiles(h):
    """(tile, row0, row1) triples (within the 20 q-tiles) contracting head h."""
    p = h // 2
    if h % 2 == 0:
        return [(5 * p + 0, 0, 128), (5 * p + 1, 0, 128), (5 * p + 2, 0, 64)]
    return [(5 * p + 3, 0, 128), (5 * p + 4, 0, 128), (5 * p + 2, 64, 128)]


def _ao_segments():
    """Per d-tile (real feature order) segments for attn@V:
    list over tiles of [(head, d0, d1, psum_base), ...]."""
    segs = [[] for _ in range(DT)]
    for h in range(H):
        d = HD * h
        end = HD * (h + 1)
        while d < end:
            nxt = min(end, (d // P + 1) * P)
            segs[d // P].append((h, d, nxt, d % P))
            d = nxt
    return segs


def _tileize_f32(wT, chunk):
    """[Kin, N] -> [N/chunk, 128, Kin/128, chunk] contiguous fp32 blocks."""
    kin, n = wT.shape
    return np.ascontiguousarray(
        wT.reshape(kin // P, P, n // chunk, chunk).transpose(2, 1, 0, 3))


def _fp8(a):
    return np.clip(a, -FP8_MAX, FP8_MAX).astype(ml_dtypes.float8_e4m3)


def build_core_program(use_qk_bias, use_v_bias, use_out_bias, use_b1, use_b2,
                       ln1_affine, ln2_affine, use_mask):
    import concourse.bass as bass
    import concourse.bacc as bacc
    import concourse.mybir as mybir
    import concourse.tile as tile
    from concourse.masks import make_identity

    F32 = mybir.dt.float32
    BF16 = mybir.dt.bfloat16
    F8 = mybir.dt.float8e4
    DR = mybir.MatmulPerfMode.DoubleRow

    V_DT = F8 if FP8_V else BF16
    OUT_DT = F8 if FP8_OUT else BF16

    k_desc = 1.0 / (SX * SW)          # q/k psum descale (both parts x1024)
    q_desc = k_desc / np.sqrt(HD)

    nc = bacc.Bacc()
    dp = nc.declare_dram_parameter
    xT8 = dp("xT8", [NHALF, P, DT, THALF], F8, isOutput=False)
    xTb = dp("xTb", [NHALF, P, DT - N8QK, THALF], BF16, isOutput=False)
    x_nat = dp("x", [TOK, D], F32, isOutput=False)
    qk_wT8 = dp("qk_wT8", [QKT, P, N8QK, P], F8, isOutput=False)
    qk_wTb = dp("qk_wTb", [QKT, P, DT - N8QK, P], BF16, isOutput=False)
    v_wT = dp("v_wT", [D // CH, P, DT, CH], V_DT, isOutput=False)
    out_wT = dp("out_wT", [D // CH, P, DT, CH], OUT_DT, isOutput=False)
    w1T = dp("w1T", [FT, P, DT, P], BF16, isOutput=False)
    w2T = dp("w2T", [D // CH, P, FT, CH], BF16, isOutput=False)
    qk_b = dp("qk_b", [2 * D], F32, isOutput=False) if use_qk_bias else None
    v_b = dp("v_b", [D], F32, isOutput=False) if use_v_bias else None
    out_b = dp("out_b", [D], F32, isOutput=False) if use_out_bias else None
    b1 = dp("b1", [DFF], F32, isOutput=False) if use_b1 else None
    b2 = dp("b2", [D], F32, isOutput=False) if use_b2 else None
    ln1_g = dp("ln1_g", [D], F32, isOutput=False) if ln1_affine else None
    ln1_b = dp("ln1_b", [D], F32, isOutput=False) if ln1_affine else None
    ln2_g = dp("ln2_g", [D], F32, isOutput=False) if ln2_affine else None
    ln2_b = dp("ln2_b", [D], F32, isOutput=False) if ln2_affine else None
    mask_in = dp("mask", [BL, K], F32, isOutput=False) if use_mask else None
    out = dp("out", [TOK, D], F32, isOutput=True)

    x1_dram = nc.dram_tensor("x1_scratch", [TOK, D], F32)

    Exp = mybir.ActivationFunctionType.Exp
    Relu = mybir.ActivationFunctionType.Relu
    Sqrt = mybir.ActivationFunctionType.Sqrt
    Copy = mybir.ActivationFunctionType.Copy
    Ident = mybir.ActivationFunctionType.Identity
    AX = mybir.AxisListType.X
    OP = mybir.AluOpType

    def bcast_dram(ap, n_part=P):
        return bass.AP(tensor=ap.tensor, offset=ap.offset,
                       ap=[[0, n_part]] + list(ap.ap))

    ao_segs = _ao_segments()

    with tile.TileContext(nc) as tc:
        with (
            tc.tile_pool(name="consts", bufs=1) as consts,
            tc.tile_pool(name="aot", bufs=1) as aot,
        ):
            id_bf = consts.tile([P, P], BF16)
            make_identity(nc, id_bf)
            id_f32 = consts.tile([P, P], F32)
            make_identity(nc, id_f32)
            # LN eps consts absorbing activation scaling: LN1 computes
            # sqrt(var_s/S2^2 + EPS*S1^2/S2^2), LN2 sqrt(var_s + EPS*S2^2)
            eps1_sb = consts.tile([P, 1], F32)
            nc.vector.memset(eps1_sb, EPS * (S1 * S1) / (S2 * S2))
            eps2_sb = consts.tile([P, 1], F32)
            nc.vector.memset(eps2_sb, EPS * (S2 * S2))

            # first out_proj weight chunk, preloaded so phase B starts hot
            wo_first = consts.tile([P, DT, CH], OUT_DT)
            nc.gpsimd.dma_start(out=wo_first, in_=out_wT[0])

            # attention output transposed, resident in SBUF (no DRAM trip)
            aoT_sb = aot.tile([P, BL, DT, P], OUT_DT)

            qkb_sb = None
            if use_qk_bias:
                qkb_sb = consts.tile([P, QKT], F32)
                nc.sync.dma_start(out=qkb_sb,
                                  in_=qk_b[:].rearrange("(t p) -> p t", p=P))
            vb_sb = None
            if use_v_bias:
                # v_b host-scaled by VRAW (v_sb carries VRAW*v)
                vb_sb = consts.tile([P, D], F32)
                nc.gpsimd.dma_start(out=vb_sb, in_=bcast_dram(v_b[:]))
            outb_sb = None
            if use_out_bias:
                outb_sb = consts.tile([P, D], F32)
                nc.gpsimd.dma_start(out=outb_sb, in_=bcast_dram(out_b[:]))
            b1_sb = None
            if use_b1:
                b1_sb = consts.tile([P, FT], F32)
                nc.sync.dma_start(out=b1_sb,
                                  in_=b1[:].rearrange("(t p) -> p t", p=P))
            b2_sb = None
            if use_b2:
                b2_sb = consts.tile([P, D], F32)
                nc.gpsimd.dma_start(out=b2_sb, in_=bcast_dram(b2[:]))
            ln1g_sb = ln1b_sb = ln2g_sb = ln2b_sb = None
            if ln1_affine:
                ln1g_sb = consts.tile([P, D], F32)
                nc.gpsimd.dma_start(out=ln1g_sb, in_=bcast_dram(ln1_g[:]))
                ln1b_sb = consts.tile([P, D], F32)
                nc.gpsimd.dma_start(out=ln1b_sb, in_=bcast_dram(ln1_b[:]))
            if ln2_affine:
                ln2g_sb = consts.tile([P, D], F32)
                nc.gpsimd.dma_start(out=ln2g_sb, in_=bcast_dram(ln2_g[:]))
                ln2b_sb = consts.tile([P, D], F32)
                nc.gpsimd.dma_start(out=ln2b_sb, in_=bcast_dram(ln2_b[:]))
            mask_sb = None
            if use_mask:
                mask_sb = consts.tile([P, BL, K], F32)
                nc.gpsimd.dma_start(
                    out=mask_sb, in_=bcast_dram(mask_in[:, :]))

            # ======== attention: both halves share one set of buffers ========
            with (
                tc.tile_pool(name="attn_sb", bufs=1) as asb,
                tc.tile_pool(name="aw", bufs=3) as aw,
                tc.tile_pool(name="bt", bufs=2) as bt,
            ):
                xT8_sb = asb.tile([P, DT, THALF], F8)
                xTb_sb = asb.tile([P, DT - N8QK, THALF], BF16)
                v_sb = asb.tile([P, BHALF, D], BF16)
                qkT_sb = asb.tile([P, QKT, THALF], BF16)

                for half in range(NHALF):
                    nc.sync.dma_start(out=xT8_sb, in_=xT8[half])
                    nc.sync.dma_start(out=xTb_sb, in_=xTb[half])

                    with tc.tile_pool(name=f"aps{half}", bufs=4,
                                      space="PSUM") as aps:
                        # V projection: natural [tok, vfeat]; v_sb holds the
                        # RAW psum (xVRAW); descale rides the attnT copy.
                        for c in range(D // CH):
                            wv = aw.tile([P, DT, CH], V_DT, tag="wv")
                            nc.sync.dma_start(out=wv, in_=v_wT[c])
                            for t in range(BHALF):
                                ps = aps.tile([P, CH], F32, tag="ps_a")
                                if FP8_V:
                                    for k in range(DT // 2):
                                        nc.tensor.matmul(
                                            ps,
                                            xT8_sb[:, 2 * k:2 * k + 2,
                                                   t * P:(t + 1) * P],
                                            wv[:, 2 * k:2 * k + 2, :],
                                            start=(k == 0),
                                            stop=(k == DT // 2 - 1),
                                            perf_mode=DR)
                                else:
                                    for k in range(DT):
                                        nc.tensor.matmul(
                                            ps,
                                            xTb_sb[:, k - N8QK,
                                                   t * P:(t + 1) * P],
                                            wv[:, k, :],
                                            start=(k == 0), stop=(k == DT - 1))
                                vdst = v_sb[:, t, c * CH:(c + 1) * CH]
                                if use_v_bias:
                                    nc.vector.tensor_add(
                                        out=vdst, in0=ps,
                                        in1=vb_sb[:, c * CH:(c + 1) * CH])
                                else:
                                    nc.vector.tensor_copy(out=vdst, in_=ps)

                        # Q/K projection: transposed [feat, tok]; first N8QK
                        # k-tiles fp8-DR, rest bf16 (weights carry x1024)
                        for jt in range(QKT):
                            wq8 = aw.tile([P, N8QK, P], F8, tag="wq8")
                            nc.sync.dma_start(out=wq8, in_=qk_wT8[jt])
                            wqb = aw.tile([P, DT - N8QK, P], BF16, tag="wqb")
                            nc.sync.dma_start(out=wqb, in_=qk_wTb[jt])
                            ps = aps.tile([P, CH], F32, tag="ps_a")
                            for k in range(N8QK // 2):
                                nc.tensor.matmul(
                                    ps, wq8[:, 2 * k:2 * k + 2, :],
                                    xT8_sb[:, 2 * k:2 * k + 2, :],
                                    start=(k == 0), stop=False,
                                    perf_mode=DR)
                            for k in range(N8QK, DT):
                                nc.tensor.matmul(
                                    ps, wqb[:, k - N8QK, :],
                                    xTb_sb[:, k - N8QK, :],
                                    start=False, stop=(k == DT - 1))
                            desc = q_desc if jt < DT else k_desc
                            if use_qk_bias:
                                nc.scalar.activation(
                                    out=qkT_sb[:, jt, :], in_=ps, func=Ident,
                                    bias=qkb_sb[:, jt:jt + 1], scale=desc)
                            else:
                                nc.scalar.activation(out=qkT_sb[:, jt, :],
                                                     in_=ps, func=Copy,
                                                     scale=desc)

                    # attention per batch: scores -> transposes -> attn@V,
                    # each stage contiguous on PE so no mid-stream waits
                    with (
                        tc.tile_pool(name=f"sps{half}", bufs=4,
                                     space="PSUM") as sps,
                        tc.tile_pool(name=f"tps{half}", bufs=2,
                                     space="PSUM") as tps,
                        tc.tile_pool(name=f"ops{half}", bufs=2,
                                     space="PSUM") as ops,
                    ):
                        for bi in range(BHALF):
                            b = half * BHALF + bi
                            csl = slice(bi * P, (bi + 1) * P)
                            attn = bt.tile([P, H, P], BF16, tag="attn")
                            negmax = bt.tile([P, H], F32, tag="negmax")
                            esum = bt.tile([P, H], F32, tag="esum")
                            rinv = bt.tile([P, H], F32, tag="rinv")
                            attnT = bt.tile([P, H, P], BF16, tag="attnT")
                            for h in range(H):
                                sc = sps.tile([P, P], F32, tag="sc")
                                kts = _score_ktiles(h)
                                for i, (t, r0, r1) in enumerate(kts):
                                    nc.tensor.matmul(
                                        sc, qkT_sb[r0:r1, t, csl],
                                        qkT_sb[r0:r1, DT + t, csl],
                                        start=(i == 0), stop=(i == len(kts) - 1))
                                nc.vector.tensor_reduce(
                                    out=negmax[:, h:h + 1], in_=sc, axis=AX,
                                    op=OP.max, negate=True)
                                nc.scalar.activation(
                                    out=attn[:, h, :], in_=sc, func=Exp,
                                    bias=negmax[:, h:h + 1], scale=1.0,
                                    accum_out=esum[:, h:h + 1])
                                if use_mask:
                                    nc.vector.tensor_mul(
                                        out=attn[:, h, :], in0=attn[:, h, :],
                                        in1=mask_sb[:, b, :])
                                    nc.vector.tensor_reduce(
                                        out=esum[:, h:h + 1], in_=attn[:, h, :],
                                        axis=AX, op=OP.add)
                                nc.vector.reciprocal(out=rinv[:, h:h + 1],
                                                     in_=esum[:, h:h + 1])
                                nc.vector.tensor_scalar_mul(
                                    out=attn[:, h, :], in0=attn[:, h, :],
                                    scalar1=rinv[:, h:h + 1])
                            for h in range(H):
                                tp = tps.tile([P, P], BF16, tag="tp")
                                nc.tensor.transpose(tp, attn[:, h, :], id_bf)
                                # attnT carries attn/VRAW: cancels v_sb's
                                # VRAW so ao psum is true-scale
                                nc.vector.tensor_scalar_mul(
                                    out=attnT[:, h, :], in0=tp,
                                    scalar1=1.0 / VRAW)
                            for t in range(DT):
                                ao = ops.tile([P, P], F32, tag="ao")
                                for (h, d0, d1, base) in ao_segs[t]:
                                    w = d1 - d0
                                    nc.tensor.matmul(
                                        ao[base:base + w, :], v_sb[:, bi, d0:d1],
                                        attnT[:, h, :], start=True, stop=True,
                                        tile_position=((0, base) if base
                                                       else None))
                                nc.scalar.activation(
                                    out=aoT_sb[:, b, t, :], in_=ao, func=Copy,
                                    scale=(SAO if FP8_OUT else 1.0))

            # ==== per group: out_proj+LN1 -> FFN1 -> FFN2+LN2 (fused B/C) ====
            # Group g's FFN2 matmuls follow g's FFN1 directly, so the
            # vector/scalar LayerNorm tails overlap the next group's matmuls
            # instead of serializing at phase end. LayerNorm apply runs on
            # the scalar engine (out = y*rstd + (-mean*rstd)).
            NGRP = 2
            TPG = BL // NGRP          # tok-tiles per group
            GW = TPG * P              # tokens per group (512)
            with (
                tc.tile_pool(name="hres", bufs=1) as hres,
                tc.tile_pool(name="csb", bufs=2) as csb,
                tc.tile_pool(name="cxr", bufs=4) as cxr,
                tc.tile_pool(name="cw", bufs=2) as cw,
                tc.tile_pool(name="cy", bufs=1) as cy,
                tc.tile_pool(name="cx1t", bufs=1) as cx1t,
                tc.tile_pool(name="dw", bufs=3) as dw,
                tc.tile_pool(name="ew", bufs=2) as ew,
                tc.tile_pool(name="cps", bufs=6, space="PSUM") as cps,
                tc.tile_pool(name="ctps", bufs=2, space="PSUM") as ctps,
            ):
                hT = hres.tile([P, FT, TOK], BF16)
                # one resident y buffer: out_proj writes it, LN1 reads it,
                # FFN2 overwrites it (per-slice deps order the reuse)
                y_grp = cy.tile([P, TPG, D], F32)

                def ln_apply(stats_row, eps_sb, sqrt_scale, out_t, y_row,
                             gain_sb, bias_sb):
                    mv = csb.tile([P, 2], F32, tag="mv")
                    nc.vector.bn_aggr(out=mv, in_=stats_row)
                    std = csb.tile([P, 1], F32, tag="std")
                    nc.scalar.activation(out=std, in_=mv[:, 1:2],
                                         func=Sqrt, bias=eps_sb,
                                         scale=sqrt_scale)
                    rstd = csb.tile([P, 1], F32, tag="rstd")
                    nc.vector.reciprocal(out=rstd, in_=std)
                    nc.vector.tensor_scalar(out=out_t, in0=y_row,
                                            scalar1=mv[:, 0:1], scalar2=rstd,
                                            op0=OP.subtract, op1=OP.mult)
                    if gain_sb is not None:
                        nc.vector.tensor_mul(out=out_t, in0=out_t, in1=gain_sb)
                        nc.vector.tensor_add(out=out_t, in0=out_t, in1=bias_sb)

                # one-chunk-ahead prefetch rings for the streamed weights so
                # group boundaries never expose the DMA latency
                NCH = D // CH
                wo_seq = [c for _ in range(NGRP) for c in range(NCH)]
                w2_seq = list(wo_seq)

                def wo_fetch(i):
                    if i >= len(wo_seq):
                        return None
                    t = cw.tile([P, DT, CH], OUT_DT, tag="wo")
                    nc.sync.dma_start(out=t, in_=out_wT[wo_seq[i]])
                    return t

                def w2_fetch(i):
                    if i >= len(w2_seq):
                        return None
                    t = ew.tile([P, FT, CH], BF16, tag="w2c")
                    nc.sync.dma_start(out=t, in_=w2T[w2_seq[i]])
                    return t

                wo_cur, wo_next = wo_first, wo_fetch(1)
                w2_cur, w2_next = w2_fetch(0), w2_fetch(1)

                for g in range(NGRP):
                    stats_g = csb.tile([P, TPG, 5, 6], F32, tag="stats")
                    x1T_grp = cx1t.tile([P, DT, GW], BF16, tag="x1T_grp")
                    # ---- out_proj + residual + LN1 ----
                    for c in range(D // CH):
                        wo = wo_cur
                        wo_cur, wo_next = wo_next, wo_fetch(g * NCH + c + 2)
                        for ti in range(TPG):
                            tt = g * TPG + ti
                            ps = cps.tile([P, CH], F32, tag="ps")
                            if FP8_OUT:
                                for k in range(DT // 2):
                                    nc.tensor.matmul(
                                        ps,
                                        aoT_sb[:, tt, 2 * k:2 * k + 2, :],
                                        wo[:, 2 * k:2 * k + 2, :],
                                        start=(k == 0),
                                        stop=(k == DT // 2 - 1),
                                        perf_mode=DR)
                            else:
                                for k in range(DT):
                                    nc.tensor.matmul(
                                        ps, aoT_sb[:, tt, k, :], wo[:, k, :],
                                        start=(k == 0), stop=(k == DT - 1))
                            if use_out_bias:
                                # out_b host-scaled by S1
                                nc.vector.tensor_add(
                                    out=ps, in0=ps,
                                    in1=outb_sb[:, c * CH:(c + 1) * CH])
                            # deep ring: the trigger's WAR wait looks 4 allocs
                            # back, so it never stalls the sync DMA queue
                            xr = cxr.tile([P, CH], F32, tag="xr")
                            nc.sync.dma_start(
                                out=xr,
                                in_=x_nat[tt * P:(tt + 1) * P,
                                          c * CH:(c + 1) * CH])
                            nc.vector.tensor_add(
                                out=y_grp[:, ti, c * CH:(c + 1) * CH],
                                in0=ps, in1=xr)
                            nc.vector.bn_stats(
                                out=stats_g[:, ti, c, :],
                                in_=y_grp[:, ti, c * CH:(c + 1) * CH])
                            if c != D // CH - 1:
                                continue
                            # ---- LN1(ti), overlaps next tile's matmuls ----
                            x1_t = csb.tile([P, D], F32, tag="x1t")
                            ln_apply(stats_g[:, ti], eps1_sb,
                                     1.0 / (S2 * S2), x1_t, y_grp[:, ti, :],
                                     ln1g_sb if ln1_affine else None, ln1b_sb)
                            # gpsimd queue: this write waits on vector compute
                            # and must not block weight loads on sync's queue
                            nc.gpsimd.dma_start(
                                out=x1_dram[tt * P:(tt + 1) * P, :],
                                in_=x1_t)
                            for k in range(DT):
                                tp = ctps.tile([P, P], F32, tag="tp_c")
                                nc.tensor.transpose(
                                    tp, x1_t[:, k * P:(k + 1) * P], id_f32)
                                nc.scalar.activation(
                                    out=x1T_grp[:, k, ti * P:(ti + 1) * P],
                                    in_=tp, func=Copy, scale=1.0 / S2)
                    # ---- FFN1 (relu, output into hT) ----
                    for ft in range(FT):
                        w1 = dw.tile([P, DT, P], BF16, tag="w1")
                        nc.sync.dma_start(out=w1, in_=w1T[ft])
                        ps = cps.tile([P, CH], F32, tag="ps")
                        for k in range(DT):
                            nc.tensor.matmul(
                                ps, w1[:, k, :], x1T_grp[:, k, :],
                                start=(k == 0), stop=(k == DT - 1))
                        osl = slice(g * GW, (g + 1) * GW)
                        if use_b1:
                            nc.scalar.activation(
                                out=hT[:, ft, osl], in_=ps, func=Relu,
                                bias=b1_sb[:, ft:ft + 1], scale=1.0)
                        else:
                            nc.scalar.activation(out=hT[:, ft, osl],
                                                 in_=ps, func=Relu)
                    # ---- FFN2 + residual + LN2 (reuses y_grp) ----
                    stats_e = csb.tile([P, TPG, 5, 6], F32, tag="stats_e")
                    for c in range(D // CH):
                        w2c = w2_cur
                        w2_cur, w2_next = w2_next, w2_fetch(g * NCH + c + 2)
                        for ti in range(TPG):
                            tt = g * TPG + ti
                            ps = cps.tile([P, CH], F32, tag="ps")
                            for k in range(FT):
                                nc.tensor.matmul(
                                    ps, hT[:, k, tt * P:(tt + 1) * P],
                                    w2c[:, k, :],
                                    start=(k == 0), stop=(k == FT - 1))
                            if use_b2:
                                # b2 host-scaled by S2
                                nc.vector.tensor_add(
                                    out=ps, in0=ps,
                                    in1=b2_sb[:, c * CH:(c + 1) * CH])
                            xr = cxr.tile([P, CH], F32, tag="xr")
                            nc.sync.dma_start(
                                out=xr,
                                in_=x1_dram[tt * P:(tt + 1) * P,
                                            c * CH:(c + 1) * CH])
                            nc.vector.tensor_add(
                                out=y_grp[:, ti, c * CH:(c + 1) * CH],
                                in0=ps, in1=xr)
                            nc.vector.bn_stats(
                                out=stats_e[:, ti, c, :],
                                in_=y_grp[:, ti, c * CH:(c + 1) * CH])
                            if c != D // CH - 1:
                                continue
                            # ---- LN2(ti), overlaps next tile's matmuls ----
                            o_t = csb.tile([P, D], F32, tag="x1t")
                            ln_apply(stats_e[:, ti], eps2_sb, 1.0, o_t,
                                     y_grp[:, ti, :],
                                     ln2g_sb if ln2_affine else None, ln2b_sb)
                            nc.gpsimd.dma_start(
                                out=out[tt * P:(tt + 1) * P, :], in_=o_t)

    nc.compile()
    return nc


def _prep_inputs(x, distances, mask, qkv_w, qkv_b, out_w, out_b,
                 bias_w1, bias_b1, bias_w2, bias_b2,
                 ffn_w1, ffn_b1, ffn_w2, ffn_b2,
                 ln1_g, ln1_b, ln2_g, ln2_b):
    """Host-side shard + weight formatting. Returns (flags, in_maps)."""
    bf16 = ml_dtypes.bfloat16
    perm = _qk_perm()

    # q/k: no sqrt(HD) fold (descale handles it); bf16 part carries x1024
    # so both halves of the contraction accumulate at the same psum scale
    q_w = qkv_w[0:D][perm]
    k_w = qkv_w[D:2 * D][perm]
    qk_cat = np.concatenate([q_w, k_w], axis=0).T.astype(np.float32)
    qk_tz = _tileize_f32(qk_cat, P)                 # [40, P, DT, P]
    qk_wT8 = _fp8(qk_tz[:, :, :N8QK] * np.float32(SW))
    qk_wTb = (qk_tz[:, :, N8QK:] * np.float32(SX * SW)).astype(bf16)

    v_tz = _tileize_f32(qkv_w[2 * D:3 * D].T.astype(np.float32), CH)
    if FP8_V:
        v_wT = _fp8(v_tz * np.float32(SW))
    else:
        v_wT = v_tz.astype(bf16)

    o_tz = _tileize_f32(out_w.T.astype(np.float32), CH)
    if FP8_OUT:
        out_wT = _fp8(o_tz * np.float32(SW))
    else:
        out_wT = o_tz.astype(bf16)
    w1T = _tileize_f32(ffn_w1.T.astype(np.float32), P).astype(bf16)
    w2T = _tileize_f32(ffn_w2.T.astype(np.float32), CH).astype(bf16)

    qk_b = np.concatenate([qkv_b[0:D][perm] * np.float32(1.0 / np.sqrt(HD)),
                           qkv_b[D:2 * D][perm]]).astype(np.float32)
    v_b = np.ascontiguousarray(qkv_b[2 * D:3 * D]).astype(np.float32)

    flags = dict(
        use_qk_bias=bool(np.any(qk_b != 0)),
        use_v_bias=bool(np.any(v_b != 0)),
        use_out_bias=bool(np.any(out_b != 0)),
        use_b1=bool(np.any(ffn_b1 != 0)),
        use_b2=bool(np.any(ffn_b2 != 0)),
        ln1_affine=not (np.all(ln1_g == 1) and np.all(ln1_b == 0)),
        ln2_affine=not (np.all(ln2_g == 1) and np.all(ln2_b == 0)),
        use_mask=not bool(np.all(mask)),
    )

    shared = {"qk_wT8": qk_wT8, "qk_wTb": qk_wTb, "v_wT": v_wT,
              "out_wT": out_wT, "w1T": w1T, "w2T": w2T}
    if flags["use_qk_bias"]:
        shared["qk_b"] = qk_b
    if flags["use_v_bias"]:
        shared["v_b"] = (v_b * np.float32(VRAW)).astype(np.float32)
    if flags["use_out_bias"]:
        shared["out_b"] = (out_b * np.float32(S1)).astype(np.float32)
    if flags["use_b1"]:
        shared["b1"] = ffn_b1.astype(np.float32)
    if flags["use_b2"]:
        shared["b2"] = (ffn_b2 * np.float32(S2)).astype(np.float32)
    if flags["ln1_affine"]:
        shared["ln1_g"] = ln1_g.astype(np.float32)
        shared["ln1_b"] = (ln1_b * np.float32(S2)).astype(np.float32)
    if flags["ln2_affine"]:
        shared["ln2_g"] = ln2_g.astype(np.float32)
        shared["ln2_b"] = ln2_b.astype(np.float32)

    in_maps = []
    for c in range(NCORES):
        xc = np.ascontiguousarray(
            x[c * BL:(c + 1) * BL].reshape(TOK, D)).astype(np.float32)
        xcT = xc.T                                   # [D, TOK]
        xT8_blocks = np.ascontiguousarray(
            _fp8(xcT * np.float32(SX)).reshape(
                DT, P, NHALF, THALF).transpose(2, 1, 0, 3))
        xTb_blocks = np.ascontiguousarray(
            xcT.astype(bf16).reshape(
                DT, P, NHALF, THALF)[N8QK:].transpose(2, 1, 0, 3))
        m = {"x": xc * np.float32(S1), "xT8": xT8_blocks, "xTb": xTb_blocks,
             **shared}
        if flags["use_mask"]:
            m["mask"] = mask[c * BL:(c + 1) * BL].astype(np.float32)
        in_maps.append(m)
    return flags, in_maps


def run(trace=False, **inputs):
    """Build + run on 8 cores. Returns (output, BassKernelResults)."""
    from concourse.bass_utils import run_bass_kernel_spmd

    inputs = {k: np.asarray(v) for k, v in inputs.items()}
    flags, in_maps = _prep_inputs(**inputs)
    nc = build_core_program(**flags)
    res = run_bass_kernel_spmd(nc, in_maps, list(range(NCORES)), trace=trace)
    out = np.stack([np.asarray(res.results[c]["out"], dtype=np.float32)
                    for c in range(NCORES)])
    return out.reshape(B, K, D), res


def kernel(**inputs):
    out, _ = run(trace=False, **inputs)
    return out
